# revision 35
# baseline (speedup 1.0000x reference)
"""Causal multi-head attention (B=2, S=2048, D=2048, 32 heads x 64) for 8
Trainium2 NeuronCores.

Sharding: data parallel on batch (2 groups of 4 cores) x tensor parallel on
heads (4 groups of 8 heads each). Each core computes q/k/v projections for
its head group, RoPE, causal attention with sigmoid-gated values, and a
partial o-projection; the host sums the 4 partials per batch (the
"all-reduce" of the o-projection) and adds the output bias + gate-mean
constant.

Design (~481us HW exec vs the 608us f32r baseline):
- All matmul operands bf16 (1 cyc/row at any N); PSUM accumulates fp32;
  output partials ship bf16 (summed in fp32 on the host).
- PSUM pools stay open across phases (pool transitions drain the PE and
  cost a DVFS re-ramp: ~3us of half-clock matmuls after any PE gap); the
  projection/V tiles share one 2-bank tag. Only one transition exists,
  into the qb2/qb3 attention pools.
- x is loaded once per half; the V projection reuses the resident pair,
  two seq-tiles per PSUM pair with a single strided tanh.
- Software-pipelined interleaving keeps the PE dense: attention for
  query blocks 0-1 (needs only half-0 q/k/v) is woven into the half-1
  projection stream at sub-unit granularity; o-projections of finished
  blocks are spread proportionally through the next block's attention;
  AV matmuls trail their exp by two pairs.
- RoPE uses a host-side d-permutation (pairs (d, d+32) on adjacent
  partitions; scores are invariant to the shared q/k permutation) so
  rotate-half is a within-quadrant stream_shuffle lane swap and every
  DVE op is full-width and partition-aligned. Weights, biases and
  cos/sin tables are permuted to match on the host.
- Gate = tanh(v/2) = 2*sigmoid(v)-1: the 0.5 folds into Wo and the +0.5
  mean term becomes a host constant; tanh shares the exp activation
  table (zero table loads after a warmup) and centering halves the bf16
  quantization error of the gated values.
- Scores contract 64 partitions directly; causal masking is a PE
  identity-matmul accumulation into the scores group (width-sliced);
  softmax-exp covers two key tiles per activation ([128,2,512] PSUM
  pair -> bf16); the denominator rides the AV matmul as a ones column.
"""

import os

import numpy as np
import ml_dtypes

import concourse.bacc as bacc
import concourse.tile as tile
from concourse import mybir
from concourse.bass_utils import run_bass_kernel_spmd

B, S, D = 2, 2048, 2048
H_PER_CORE = 8          # heads per core
DH = 64                 # head dim
CW = 512                # per-core projection width = H_PER_CORE * DH
N_CORES = 8
KT = D // 128           # k-subtiles for the D-contraction

f32 = mybir.dt.float32
bf16 = mybir.dt.bfloat16
Act = mybir.ActivationFunctionType

TRACE = bool(int(os.environ.get("KERNEL_TRACE", "0")))
LAST_EXEC_NS = None
LAST_MEAN_NS = None

_SENT = object()
_SWAP_MASK = [i ^ 1 for i in range(32)]


def _build(WITH_BIAS=True):
    nc = bacc.Bacc("TRN2", target_bir_lowering=False, debug=False)

    x4 = nc.dram_tensor("x4", [4, 128, KT, 512], bf16, kind="ExternalInput")
    wq4 = nc.dram_tensor("wq4", [4, 128, KT, 128], bf16, kind="ExternalInput")
    wk4 = nc.dram_tensor("wk4", [4, 128, KT, 128], bf16, kind="ExternalInput")
    wv4 = nc.dram_tensor("wv4", [128, KT, CW], bf16, kind="ExternalInput")
    wo4 = nc.dram_tensor("wo4", [4, 128, 4, 512], bf16, kind="ExternalInput")
    bq = nc.dram_tensor("bq", [1, CW], bf16, kind="ExternalInput")
    bk = nc.dram_tensor("bk", [1, CW], bf16, kind="ExternalInput")
    bv = nc.dram_tensor("bv", [1, CW], bf16, kind="ExternalInput")
    ropec = nc.dram_tensor("ropec", [4, 128, 512], f32, kind="ExternalInput")
    ropes = nc.dram_tensor("ropes", [4, 128, 512], f32, kind="ExternalInput")
    masks = nc.dram_tensor("masks", [128, 4, 512], bf16, kind="ExternalInput")
    ident = nc.dram_tensor("ident", [128, 128], bf16, kind="ExternalInput")
    vinit = nc.dram_tensor("vinit", [128, 16 * 520], bf16, kind="ExternalInput")
    part = nc.dram_tensor("part", [S, D], bf16, kind="ExternalOutput")

    with tile.TileContext(nc) as tc:
        with (
            tc.tile_pool(name="p0", bufs=1) as p0,
            tc.tile_pool(name="pqk", bufs=1) as pqk,
            tc.tile_pool(name="py", bufs=1) as py,
            tc.tile_pool(name="pa", bufs=1) as pa,
            tc.tile_pool(name="paw", bufs=5) as paw,
            tc.tile_pool(name="prc", bufs=1) as prc,
            tc.tile_pool(name="prt", bufs=2) as prt,
            tc.tile_pool(name="pba", bufs=4) as pba,
            tc.tile_pool(name="pbs", bufs=2) as pbs,
            tc.tile_pool(name="pc", bufs=2) as pc,
            tc.tile_pool(name="pbo", bufs=2) as pbo,
        ):
            # persistent state
            qt_all = pqk.tile([128, 4, S], bf16, name="qt_all")
            kt_all = pqk.tile([128, 4, S], bf16, name="kt_all")
            qt = [qt_all[:, i, :] for i in range(4)]
            kt = [kt_all[:, i, :] for i in range(4)]
            va_all = p0.tile([128, 16 * 520 + 4 * CW], bf16, name="va_all")
            va = [va_all[:, 520 * i:520 * (i + 1)] for i in range(16)]
            ones = va_all[0:1, 8320:8320 + CW]
            bvt = va_all[0:1, 8832:8832 + CW]
            bqrow = va_all[0:1, 9344:9344 + CW]
            bkrow = va_all[0:1, 9856:9856 + CW]
            idt = p0.tile([128, 128], bf16, name="idt")
            maskt = p0.tile([128, 4, 512], bf16, name="maskt")
            wvf = p0.tile([128, KT, CW], bf16, name="wvf")
            ytr = [py.tile([128, S], bf16, name=f"ytr{i}") for i in range(4)]

            warm = p0.tile([1, 8], f32, name="warm")

            def load_xh(half, first_wch=None):
                xh = pa.tile([128, 2, KT, 512], bf16, tag="xh", name="xh")
                if first_wch is not None:
                    first_wch()
                for kg in range(4):
                    for qloc in range(2):
                        nc.sync.dma_start(
                            xh[:, qloc, 4 * kg:4 * kg + 4, :],
                            x4[2 * half + qloc, :, 4 * kg:4 * kg + 4, :])
                return xh

            def load_rope(half):
                cosw = prc.tile([128, 2, 512], f32, tag="tblc", name="cosw")
                rsnw = prc.tile([128, 2, 512], f32, tag="tbls", name="rsnw")
                for qloc in range(2):
                    nc.sync.dma_start(cosw[:, qloc, :], ropec[2 * half + qloc])
                    nc.sync.dma_start(rsnw[:, qloc, :], ropes[2 * half + qloc])
                return cosw, rsnw

            def emit_consts():
                nc.sync.dma_start(idt[:], ident[:])
                nc.sync.dma_start(maskt[:], masks[:])
                if WITH_BIAS:
                    nc.sync.dma_start(bvt, bv[:])
                    nc.sync.dma_start(bqrow, bq[:])
                    nc.sync.dma_start(bkrow, bk[:])

            def emit_vconsts():
                # needed only from the V-projection phase onward
                for kg in range(4):
                    nc.sync.dma_start(wvf[:, 4 * kg:4 * kg + 4, :],
                                      wv4[:, 4 * kg:4 * kg + 4, :])
                # va default 1.0 -> per-head 65th column stays 1 (softmax
                # denominator rides the AV matmul)
                for vg in range(4):
                    nc.sync.dma_start(
                        va_all[:, vg * 2080:(vg + 1) * 2080],
                        vinit[:, vg * 2080:(vg + 1) * 2080])

            def load_wch(w3, mt):
                wch = paw.tile([128, KT, 128], bf16, tag="wch", name="wch")
                nc.sync.dma_start(wch[:], w3[mt])
                return wch

            def qk_unit(wch, dall, brow, mt, half, xh, cosw, rsnw, pmain,
                        hook=None):
                ps = pmain.tile([128, 2, 512], f32, tag="psa", name="ps_a")
                for k in range(KT):
                    if k == 8 and hook is not None:
                        hook()
                    for qloc in range(2):
                        # qloc inner: consecutive matmuls share the
                        # stationary weight tile
                        nc.tensor.matmul(
                            ps[:, qloc, :], wch[:, k, :], xh[:, qloc, k, :],
                            start=(k == 0),
                            stop=(k == KT - 1 and not WITH_BIAS),
                        )
                if WITH_BIAS:
                    for qloc in range(2):
                        nc.tensor.matmul(
                            ps[:, qloc, :],
                            brow[:, mt * 128:(mt + 1) * 128],
                            ones, start=False, stop=True,
                        )
                # RoPE with host-interleaved d-order (pairs (d, d+32) sit on
                # adjacent partitions; scores are invariant to the shared
                # q/k permutation): rotate-half becomes a within-quadrant
                # stream_shuffle lane swap, so every op is full-width and
                # partition-aligned.
                d3 = dall[:, mt, half * 1024:(half + 1) * 1024
                          ].rearrange("p (a b) -> p a b", a=2)
                tmp = prt.tile([128, 2, 512], bf16, tag="tmp", name="tmp")
                tmp2 = prt.tile([128, 2, 512], bf16, tag="tmp2", name="tmp2")
                nc.vector.tensor_mul(tmp[:], ps[:], rsnw[:])
                nc.vector.stream_shuffle(tmp2[:], tmp[:], _SWAP_MASK)
                nc.vector.tensor_mul(d3[:], ps[:], cosw[:])
                nc.vector.tensor_add(d3[:], d3[:], tmp2[:])

            def v_unit(qloc, sp, half, xh, pmain, hook=None):
                # two seq tiles (st = 2*sp, 2*sp+1) per PSUM pair
                qtr = 2 * half + qloc
                stg0 = qtr * 4 + 2 * sp
                psb = pmain.tile([128, 2, 512], f32, tag="psa", name="ps_v")
                for j in range(2):
                    if j == 1 and hook is not None:
                        hook()
                    st = 2 * sp + j
                    for k in range(KT):
                        nc.tensor.matmul(
                            psb[:, j, :],
                            xh[:, qloc, k, st * 128:(st + 1) * 128],
                            wvf[:, k, :],
                            start=(k == 0),
                            stop=(k == KT - 1 and not WITH_BIAS),
                        )
                    if WITH_BIAS:
                        nc.tensor.matmul(
                            psb[:, j, :], ones[:, 0:128], bvt,
                            start=False, stop=True,
                        )
                # gate = tanh(v/2) = 2*sigmoid(v)-1 (0.5 folded into Wo,
                # +0.5 mean term added on host). One strided activation
                # covers both seq tiles x 8 heads.
                dst = va_all[:, 520 * stg0:520 * stg0 + 1040].rearrange(
                    "p (s h d) -> p s h d", s=2, h=8)[:, :, :, 0:64]
                nc.scalar.activation(
                    dst, psb[:].rearrange("p a (h d) -> p a h d", h=8),
                    Act.Tanh, scale=0.5,
                )

            def att_head(qb, pi, hh, psco, psy):
                nkt = 4 * qb + 4
                h = 2 * pi + hh
                lo, hi = hh * 64, (hh + 1) * 64
                yps = psy.tile([65, 512], f32, tag="yps", name="ps_y")

                def _av(at2, kp):
                    for j in range(2):
                        k_i = 2 * kp + j
                        nc.tensor.matmul(
                            yps[:], va[k_i][:, 65 * h:65 * h + 65],
                            at2[:, j, :],
                            start=(k_i == 0), stop=(k_i == nkt - 1),
                        )

                pipe = []
                for kp in range(nkt // 2):
                    ps2 = psco.tile([128, 2, 512], f32, tag="pss",
                                    name="ps_s")
                    for j in range(2):
                        k_i = 2 * kp + j
                        dt_i = k_i - 4 * qb
                        nc.tensor.matmul(
                            ps2[:, j, :],
                            kt[pi][lo:hi, k_i * 128:(k_i + 1) * 128],
                            qt[pi][lo:hi, qb * 512:(qb + 1) * 512],
                            start=True, stop=(dt_i < 0),
                        )
                        if dt_i >= 0:
                            w = 128 * (dt_i + 1)
                            nc.tensor.matmul(
                                ps2[:, j, 0:w], idt[:], maskt[:, dt_i, 0:w],
                                start=False, stop=True,
                            )
                    at2 = pba.tile([128, 2, 512], bf16, tag="at", name="at2")
                    nc.scalar.activation(at2[:], ps2[:], Act.Exp)
                    if len(pipe) == 2:
                        _av(*pipe.pop(0))
                        yield
                    pipe.append((at2, kp))
                for item in pipe:
                    _av(*item)
                    yield
                den = pbs.tile([1, 512], f32, tag="den", name="den")
                nc.vector.tensor_copy(den[:], yps[64:65, :])
                rc = pbs.tile([1, 512], f32, tag="rc", name="rc")
                nc.vector.reciprocal_approx_fast(rc[:], den[:])
                s128 = pbs.tile([128, 512], f32, tag="s128", name="s128")
                nc.gpsimd.partition_broadcast(s128[:], rc[:])
                nc.vector.tensor_mul(
                    ytr[pi][lo:hi, qb * 512:(qb + 1) * 512],
                    yps[0:64, :], s128[lo:hi, :],
                )

            def att_qbs(qbs, psco, psy):
                for qb in qbs:
                    for pi in range(4):
                        for hh in range(2):
                            yield from att_head(qb, pi, hh, psco, psy)

            def oproj_gen(qb, pso):
                for nt in range(4):
                    woc = pc.tile([128, 4, 512], bf16, tag="woc", name="woc")
                    nc.sync.dma_start(woc[:], wo4[nt])
                    for sl in range(4):
                        st = 4 * qb + sl
                        ps = pso.tile([128, 512], f32, tag="pso", name="ps_o")
                        for kc in range(4):
                            nc.tensor.matmul(
                                ps[:], ytr[kc][:, st * 128:(st + 1) * 128],
                                woc[:, kc, :],
                                start=(kc == 0), stop=(kc == 3),
                            )
                        ostg = pbo.tile([128, 512], bf16, tag="ostg",
                                        name="ostg")
                        # DVE, not Act: staging on the scalar engine queues
                        # behind the exp backlog and strangles o-proj
                        nc.vector.tensor_copy(ostg[:], ps[:])
                        nc.sync.dma_start(
                            part[st * 128:(st + 1) * 128,
                                 nt * 512:(nt + 1) * 512],
                            ostg[:],
                        )
                        yield

            def drive(gen, filler, nf, npair):
                """Pull the attention generator, spreading `nf` filler steps
                proportionally across `npair` attention pairs."""
                n = 0
                pulled = 0
                for _ in gen:
                    n += 1
                    while pulled < n * nf // npair:
                        if next(filler, _SENT) is _SENT:
                            pulled = nf
                            break
                        pulled += 1
                for _ in filler:
                    pass

            wsets = ((wq4, qt_all, bqrow), (wk4, kt_all, bkrow))

            with (
                tc.tile_pool(name="pmain", bufs=2, space="PSUM") as pmain,
                tc.tile_pool(name="psco", bufs=1, space="PSUM") as psco,
                tc.tile_pool(name="psy1", bufs=2, space="PSUM") as psy1,
            ):
                # ------------- half 0: q/k/v projections -------------
                wchq = [None] * 5

                def _first_wch():
                    wchq[0] = load_wch(wq4, 0)

                nc.vector.memset(warm[:], 0.0)
                nc.scalar.activation(warm[:], warm[:], Act.Exp)
                xh0 = load_xh(0, _first_wch)
                cosw0, rsnw0 = load_rope(0)
                for i in range(1, 4):
                    wchq[i] = load_wch(wq4, i)
                wchq[4] = load_wch(wk4, 0)
                emit_consts()
                for wi, (w3, dall, brow) in enumerate(wsets):
                    for mt in range(4):
                        pre = 4 * wi + mt
                        wch = wchq[pre] if pre < 5 else load_wch(w3, mt)
                        if pre == 2:
                            emit_vconsts()
                        qk_unit(wch, dall, brow, mt, 0, xh0, cosw0, rsnw0,
                                pmain)
                for qloc in range(2):
                    for sp in range(2):
                        v_unit(qloc, sp, 0, xh0, pmain)

                # ---- half 1 interleaved with attention qb0/qb1 ----
                xh1 = load_xh(1)
                cosw1, rsnw1 = load_rope(1)
                g01 = att_qbs((0, 1), psco, psy1)

                def take(n):
                    for _ in range(n):
                        if next(g01, _SENT) is _SENT:
                            return

                for w3, dall, brow in wsets:
                    for mt in range(4):
                        take(2)
                        qk_unit(load_wch(w3, mt), dall, brow, mt, 1, xh1,
                                cosw1, rsnw1, pmain, hook=lambda: take(2))
                for qloc in range(2):
                    for sp in range(2):
                        take(2)
                        v_unit(qloc, sp, 1, xh1, pmain,
                               hook=lambda: take(2))
                for _ in g01:
                    pass

            # ---- attention qb2/qb3 with o-proj of finished blocks ----
            with (
                tc.tile_pool(name="pssB", bufs=2, space="PSUM") as pssB,
                tc.tile_pool(name="psyB", bufs=2, space="PSUM") as psyB,
                tc.tile_pool(name="pso", bufs=2, space="PSUM") as pso,
            ):
                import itertools
                f01 = itertools.chain(oproj_gen(0, pso), oproj_gen(1, pso))
                drive(att_qbs((2,), pssB, psyB),
                      itertools.islice(f01, 24), 24, 48)
                drive(att_qbs((3,), pssB, psyB),
                      itertools.chain(f01, oproj_gen(2, pso)), 24, 64)
                for _ in oproj_gen(3, pso):
                    pass

    nc.compile()
    return nc


def _rope_tables():
    half = DH // 2
    inv_freq = 1.0 / (10000.0 ** (np.arange(0, half, dtype=np.float32) / half))
    t = np.arange(S, dtype=np.float32)
    freqs = np.einsum("i,j->ij", t, inv_freq)            # [S, 32]
    emb = np.concatenate([freqs, freqs], axis=-1)        # [S, 64]
    cos = np.cos(emb).T.astype(np.float32)                        # [64, S]
    sin = np.sin(emb).T.astype(np.float32)
    rsin = np.concatenate([-sin[:32], sin[32:]], axis=0)
    return np.ascontiguousarray(np.concatenate([cos, rsin], axis=0))  # [128, S]


def _masks():
    j = np.arange(128)[:, None, None]
    dt = np.arange(4)[None, :, None]
    i = np.arange(512)[None, None, :]
    keep = (128 * dt + j) <= i
    return np.where(keep, 0.0, -1e30).astype(np.float32)  # [128, 4, 512]


def _bf(a):
    return np.ascontiguousarray(a).astype(ml_dtypes.bfloat16)


def kernel(**inputs):
    global LAST_EXEC_NS
    x = np.asarray(inputs["x"], dtype=np.float32)
    Wq = np.asarray(inputs["Wq"], dtype=np.float32)
    Wk = np.asarray(inputs["Wk"], dtype=np.float32)
    Wv = np.asarray(inputs["Wv"], dtype=np.float32)
    Wo = np.asarray(inputs["Wo"], dtype=np.float32)
    bq = np.asarray(inputs["bq"], dtype=np.float32)
    bk = np.asarray(inputs["bk"], dtype=np.float32)
    bv = np.asarray(inputs["bv"], dtype=np.float32)
    bo = np.asarray(inputs["bo"], dtype=np.float32)

    ropeT = _rope_tables()
    masks = _masks()

    with_bias = any(float(np.abs(b).max()) > 0 for b in (bq, bk, bv))
    nc = _build(WITH_BIAS=with_bias)
    # d-order interleave: RoPE partner (d, d+32) -> adjacent partitions, so
    # rotate-half is a stream_shuffle lane swap. Scores are invariant to the
    # shared q/k permutation; W columns, biases and tables permute together.
    perm64 = np.empty(64, np.int64)
    perm64[0::2] = np.arange(32)          # new 2j   <- old j
    perm64[1::2] = np.arange(32) + 32     # new 2j+1 <- old j+32
    idx512 = (np.arange(512) // 64 * 64)[:, None] + perm64[None, :]
    idx512 = (np.arange(0, 512, 64)[:, None] + perm64[None, :]).reshape(512)
    cos64 = ropeT[0:64][perm64]                       # [64, S] permuted
    cosT = np.concatenate([cos64, cos64], axis=0)     # [128, S] dup
    sin0 = -ropeT[64:96]                              # +sin rows (j<32)
    rs64 = np.empty((64, S), np.float32)
    rs64[0::2] = sin0                                 # new 2j   -> +sin_j
    rs64[1::2] = -sin0                                # new 2j+1 -> -sin_j
    rsT = np.concatenate([rs64, rs64], axis=0)        # [128, S]
    ropec = np.ascontiguousarray(
        cosT.reshape(128, 4, 512).transpose(1, 0, 2))
    ropes = np.ascontiguousarray(
        rsT.reshape(128, 4, 512).transpose(1, 0, 2))
    vinit = _bf(np.ones((128, 16 * 520), dtype=np.float32))
    ident = _bf(np.eye(128, dtype=np.float32))
    masks_b = _bf(masks)
    in_maps = []
    for c in range(N_CORES):
        b, g = c // 4, c % 4
        sl = slice(CW * g, CW * (g + 1))
        xT = x[b].T                                    # [D, S]
        x4 = _bf(xT.reshape(KT, 128, 4, 512).transpose(2, 1, 0, 3))
        wq4 = _bf(
            Wq[sl][idx512].T.reshape(KT, 128, 4, 128).transpose(2, 1, 0, 3))
        wk4 = _bf((Wk[sl][idx512].T * 0.125)
                  .reshape(KT, 128, 4, 128).transpose(2, 1, 0, 3))
        wv4 = _bf(Wv[sl].T.reshape(KT, 128, CW).transpose(1, 0, 2))
        wo4 = _bf(
            (0.5 * Wo[:, sl]).T.reshape(4, 128, 4, 512).transpose(2, 1, 0, 3))
        in_maps.append({
            "x4": x4,
            "wq4": wq4,
            "wk4": wk4,
            "wv4": wv4,
            "wo4": wo4,
            "bq": _bf(bq[sl][idx512].reshape(1, CW)),
            "bk": _bf((bk[sl][idx512] * 0.125).reshape(1, CW)),
            "bv": _bf(bv[sl].reshape(1, CW)),
            "ropec": ropec,
            "ropes": ropes,
            "vinit": vinit,
            "ident": ident,
            "masks": masks_b,
        })

    kwargs = {}
    if TRACE:
        kwargs = dict(trace=True, trace_cores=list(range(N_CORES)),
                      stitch_traces=False)
        tdir = os.environ.get("KERNEL_TRACE_DIR")
        if tdir:
            os.makedirs(tdir, exist_ok=True)
            kwargs["tmpdir"] = tdir
    global LAST_MEAN_NS
    r = run_bass_kernel_spmd(nc, in_maps, list(range(N_CORES)), **kwargs)
    LAST_EXEC_NS = r.exec_time_ns
    LAST_MEAN_NS = r.mean_exec_time_ns

    # host "all-reduce": sum the 4 head-group partials per batch, add the
    # output bias and the 0.5*rowsum(Wo) term from the centered gate.
    const = bo + 0.5 * Wo.sum(axis=1)
    out = np.empty((B, S, D), dtype=np.float32)
    for b in range(B):
        acc = r.results[4 * b]["part"].astype(np.float32).copy()
        for g in range(1, 4):
            acc += r.results[4 * b + g]["part"]
        out[b] = acc + const
    return out
